# revision 1
# baseline (speedup 1.0000x reference)
"""Trainium2 Bass kernel for nn_BaseLinearSSM.

y[b,t] = Re(C @ x_{t+1}) + D @ u[b,t] + bias,  x_{t+1} = A x_t + B u_t  (complex A,B,C)

Strategy:
  Host (fp64): eigendecompose A = V diag(w) V^-1  (cond(V) ~ 370 for this
  problem class), fold V into B/C:  Bt = V^-1 B, Ct = C V.  The recurrence
  becomes diagonal:  xt_{t+1} = w * xt_t + Bt u_t.  Writing w = rho*e^{i th},
  z_t = e^{-i th t} xt_t obeys  z_t = rho * z_{t-1} + e^{-i th t} (Bt u)_t --
  two *real* first-order scans per mode, which map 1:1 onto the DVE's native
  tensor_tensor_scan (state = data0*state + data1).

  Device (per core, batch-sharded 2 of 16):
    f = Bt^T-matmuls of u  ->  modulate by cos/sin(th*t) tables (host fp64)
    -> tensor_tensor_scan along t  ->  demodulate  ->  y = CtRe.x_r - CtIm.x_i
    + D u accumulated in one PSUM group.

  Cores are fully independent (A/B/C/D replicated); host shards u and
  gathers y.
"""

import sys

import numpy as np

if "/opt/trn_rl_repo" not in sys.path:
    sys.path.insert(0, "/opt/trn_rl_repo")

BATCH, T, IN, OUT, N = 16, 2048, 128, 128, 512
NCORES = 8
BLOCAL = BATCH // NCORES  # 2
COLS = BLOCAL * T         # 4096 columns per core, col = b*T + t
NT = N // 128             # 4 partition tiles over the state dim
BLK = 512                 # columns per pipeline block
NBLK = COLS // BLK        # 8 blocks, (b, tb) with tb in 0..3
TBLK = T // BLK           # 4 t-blocks per batch element
# blob pieces (also DMA issue order):
#   p0: ut | btr | bti          (gates the f-matmuls)
#   p1: tb0 tables              (gates the first modulate)
#   p2: dwt | ctr | cti | rho   (gates y-projection / scans)
#   p3..p5: tb1..tb3 tables
P0W = COLS + N + N
TBW = 2 * NT * BLK  # one tb's cos+sin tables
P2W = OUT + NT * OUT + NT * OUT + NT * BLK
BLOBW = P0W + P2W + TBLK * TBW

LAST_RESULT = None  # BassKernelResults of the most recent run (for profiling)

_NC_CACHE = None


def _build_nc():
    """Build the SPMD Bass program (identical on all 8 cores)."""
    from concourse import bass, mybir
    from concourse import tile

    dt = mybir.dt.float32
    op = mybir.AluOpType

    nc = bass.Bass("TRN2", target_bir_lowering=False, debug=False)

    # All inputs packed in ONE [128, W] blob -> one DMA -> one HW queue ->
    # at most one DMA sync wait on any consumer (fused fp32 LDW+MATMUL
    # supports a single sync wait).
    blob = nc.dram_tensor("blob", [128, BLOBW], dt, kind="ExternalInput")
    yout = nc.dram_tensor("y", [OUT, COLS], dt, kind="ExternalOutput")  # [o, b*T+t]

    with tile.TileContext(nc) as tc:
        with (
            tc.tile_pool(name="const", bufs=1) as cpool,
            tc.tile_pool(name="tmp", bufs=2) as tpool,
            tc.tile_pool(name="gp", bufs=1) as gpool,
            tc.tile_pool(name="zp", bufs=2) as zpool,
            tc.tile_pool(name="xr", bufs=1) as xrpool,
            tc.tile_pool(name="xi", bufs=2) as xipool,
            tc.tile_pool(name="ysb", bufs=2) as spool,
            tc.tile_pool(name="fps", bufs=6, space="PSUM") as fpool,
            tc.tile_pool(name="yps", bufs=2, space="PSUM") as ypool,
        ):
            blob_sb = cpool.tile([128, BLOBW], dt)
            # Issue order = dependency order: f-matmul inputs, first tables,
            # projection weights, remaining tables.  _legalize_multi_waits
            # keeps any resulting wait pairing legal for walrus.
            bounds = [0, P0W, P0W + TBW, P0W + TBW + P2W]
            for k in range(2, TBLK + 1):
                bounds.append(bounds[-1] + TBW)
            for a, bnd in zip(bounds[:-1], bounds[1:]):
                nc.sync.dma_start(blob_sb[:, a:bnd], blob[:, a:bnd])
            o = [0]
            def take(w):
                s = blob_sb[:, o[0]:o[0] + w]
                o[0] += w
                return s
            ut_sb = take(COLS)
            btr_sb = take(N)
            bti_sb = take(N)
            ct_tb = [[None] * NT for _ in range(TBLK)]
            st_tb = [[None] * NT for _ in range(TBLK)]
            for m in range(NT):
                ct_tb[0][m] = take(BLK)
            for m in range(NT):
                st_tb[0][m] = take(BLK)
            dwt_sb = take(OUT)
            ctr_sb = take(NT * OUT)
            cti_sb = take(NT * OUT)
            rho_sb = [take(BLK) for _ in range(NT)]
            for k in range(1, TBLK):
                for m in range(NT):
                    ct_tb[k][m] = take(BLK)
                for m in range(NT):
                    st_tb[k][m] = take(BLK)
            assert o[0] == BLOBW

            zr_prev = [None] * NT
            zi_prev = [None] * NT
            for b in range(BLOCAL):
                for tb in range(TBLK):
                    col0 = b * T + tb * BLK
                    ucols = ut_sb[:, col0:col0 + BLK]
                    xr_blk = [None] * NT
                    xi_blk = [None] * NT
                    for m in range(NT):
                        ctt = ct_tb[tb][m][:]
                        stt = st_tb[tb][m][:]
                        # f = Bt u  (complex), PSUM
                        fre = fpool.tile([128, BLK], dt, tag="f")
                        fim = fpool.tile([128, BLK], dt, tag="f")
                        nc.tensor.matmul(
                            fre[:], btr_sb[:, m * 128:(m + 1) * 128], ucols
                        )
                        nc.tensor.matmul(
                            fim[:], bti_sb[:, m * 128:(m + 1) * 128], ucols
                        )
                        # modulate: g = e^{-i th t} f
                        t1 = tpool.tile([128, BLK], dt, tag="t1")
                        t2 = tpool.tile([128, BLK], dt, tag="t2")
                        nc.vector.tensor_tensor(t1[:], ctt, fre[:], op=op.mult)
                        nc.vector.tensor_tensor(t2[:], stt, fim[:], op=op.mult)
                        gr = gpool.tile([128, BLK], dt, tag=f"gr{m}")
                        nc.vector.tensor_tensor(gr[:], t1[:], t2[:], op=op.add)
                        t3 = tpool.tile([128, BLK], dt, tag="t1")
                        t4 = tpool.tile([128, BLK], dt, tag="t2")
                        nc.vector.tensor_tensor(t3[:], ctt, fim[:], op=op.mult)
                        nc.vector.tensor_tensor(t4[:], stt, fre[:], op=op.mult)
                        gi = gpool.tile([128, BLK], dt, tag=f"gi{m}")
                        nc.vector.tensor_tensor(gi[:], t3[:], t4[:], op=op.subtract)
                        # scan: z = rho*z_prev + g along t (chained across tb)
                        zr = zpool.tile([128, BLK], dt, tag=f"zr{m}")
                        zi = zpool.tile([128, BLK], dt, tag=f"zi{m}")
                        init_r = 0.0 if tb == 0 else zr_prev[m][:, BLK - 1:BLK]
                        init_i = 0.0 if tb == 0 else zi_prev[m][:, BLK - 1:BLK]
                        nc.vector.tensor_tensor_scan(
                            zr[:], rho_sb[m][:], gr[:], init_r, op0=op.mult, op1=op.add
                        )
                        nc.vector.tensor_tensor_scan(
                            zi[:], rho_sb[m][:], gi[:], init_i, op0=op.mult, op1=op.add
                        )
                        zr_prev[m], zi_prev[m] = zr, zi
                        # demodulate: x = e^{i th t} z
                        t5 = tpool.tile([128, BLK], dt, tag="t1")
                        t6 = tpool.tile([128, BLK], dt, tag="t2")
                        nc.vector.tensor_tensor(t5[:], ctt, zr[:], op=op.mult)
                        nc.vector.tensor_tensor(t6[:], stt, zi[:], op=op.mult)
                        xr = xrpool.tile([128, BLK], dt, tag=f"xr{m}")
                        nc.vector.tensor_tensor(xr[:], t5[:], t6[:], op=op.subtract)
                        t7 = tpool.tile([128, BLK], dt, tag="t7")
                        t8 = tpool.tile([128, BLK], dt, tag="t8")
                        nc.gpsimd.tensor_tensor(t7[:], stt, zr[:], op=op.mult)
                        nc.gpsimd.tensor_tensor(t8[:], ctt, zi[:], op=op.mult)
                        xi = xipool.tile([128, BLK], dt, tag=f"xi{m}")
                        nc.gpsimd.tensor_tensor(xi[:], t7[:], t8[:], op=op.add)
                        xr_blk[m], xi_blk[m] = xr, xi
                    # y = sum_m CtRe_m^T x_r[m] + (-CtIm_m)^T x_i[m] + D^T u
                    yps = ypool.tile([128, BLK], dt, tag="y")
                    for m in range(NT):
                        nc.tensor.matmul(
                            yps[:], ctr_sb[:, m * OUT:(m + 1) * OUT], xr_blk[m][:],
                            start=(m == 0), stop=False,
                        )
                        nc.tensor.matmul(
                            yps[:], cti_sb[:, m * OUT:(m + 1) * OUT], xi_blk[m][:],
                            start=False, stop=False,
                        )
                    nc.tensor.matmul(
                        yps[:], dwt_sb[:], ucols, start=False, stop=True
                    )
                    ysb = spool.tile([128, BLK], dt, tag="ysb")
                    nc.scalar.copy(ysb[:], yps[:])
                    nc.gpsimd.dma_start(yout[:, col0:col0 + BLK], ysb[:])

    _legalize_multi_waits(nc)
    return nc


def _legalize_multi_waits(nc):
    """This walrus build accepts a single sync wait per instruction; split
    any multi-wait instruction into same-engine single-wait NoOps + the
    original carrying the last wait (program order chains them)."""
    import bass_rust
    from concourse import mybir

    uid = [0]
    for fn in nc.m.functions:
        for bb in fn.blocks:
            insts = bb.instructions
            new = []
            changed = False
            for inst in insts:
                si = inst.sync_info
                if si is not None and len(si.on_wait) > 1:
                    waits = list(si.on_wait)
                    for w in waits[:-1]:
                        uid[0] += 1
                        new.append(mybir.InstNoOp(
                            name=f"mwsplit-{uid[0]}",
                            engine=inst.engine,
                            ins=[], outs=[],
                            sync_info=bass_rust.SyncInfo(on_wait=[w], on_update=[]),
                        ))
                    inst.sync_info = bass_rust.SyncInfo(
                        on_wait=[waits[-1]], on_update=list(si.on_update)
                    )
                    changed = True
                new.append(inst)
            if changed:
                bb.instructions = new


def _host_prep(A_re, A_im, B_re, B_im, C_re, C_im, D_w):
    """fp64 eigendecomposition + transposed/modulation-table layouts."""
    A = A_re.astype(np.float64) + 1j * A_im.astype(np.float64)
    w, V = np.linalg.eig(A)
    Vinv = np.linalg.inv(V)
    Bt = Vinv @ (B_re.astype(np.float64) + 1j * B_im.astype(np.float64))  # [N, IN]
    Ct = (C_re.astype(np.float64) + 1j * C_im.astype(np.float64)) @ V     # [OUT, N]

    rho = np.abs(w)
    theta = np.angle(w)
    tg = np.arange(1, T + 1, dtype=np.float64)
    ang = np.outer(theta, tg)  # [N, T]
    cost = np.cos(ang).astype(np.float32).reshape(NT, 128, T)
    sint = np.sin(ang).astype(np.float32).reshape(NT, 128, T)
    rho_b = np.broadcast_to(
        rho.astype(np.float32).reshape(NT, 128, 1), (NT, 128, BLK)
    ).copy()

    ctrT = np.ascontiguousarray(Ct.real.T, dtype=np.float32)   # [N, OUT]
    ctiT = np.ascontiguousarray(-Ct.imag.T, dtype=np.float32)  # [N, OUT]
    # shared blob columns (everything except the leading per-core ut block),
    # all [128, w]:
    def tbpiece(k):
        cs = cost[:, :, k * BLK:(k + 1) * BLK]  # [NT, 128, BLK]
        ss = sint[:, :, k * BLK:(k + 1) * BLK]
        return [np.ascontiguousarray(cs.transpose(1, 0, 2).reshape(128, NT * BLK)),
                np.ascontiguousarray(ss.transpose(1, 0, 2).reshape(128, NT * BLK))]
    parts = [
        np.ascontiguousarray(Bt.real.T, dtype=np.float32),  # [128(i), N]
        np.ascontiguousarray(Bt.imag.T, dtype=np.float32),
    ]
    parts += tbpiece(0)
    parts += [np.ascontiguousarray(D_w.T, dtype=np.float32)]
    parts += [np.ascontiguousarray(ctrT.reshape(NT, 128, OUT).transpose(1, 0, 2)
                                   .reshape(128, NT * OUT))]
    parts += [np.ascontiguousarray(ctiT.reshape(NT, 128, OUT).transpose(1, 0, 2)
                                   .reshape(128, NT * OUT))]
    parts += [np.ascontiguousarray(rho_b.transpose(1, 0, 2).reshape(128, NT * BLK))]
    for k in range(1, TBLK):
        parts += tbpiece(k)
    return np.concatenate(parts, axis=1)  # [128, BLOBW - COLS]


def _ensure_axon_hooks():
    """Provide antenv.axon_hooks if the image lacks it (needed only for
    trace=True NTFF profiling; run path works without)."""
    import types
    try:
        from antenv import axon_hooks  # noqa: F401
        return
    except ImportError:
        pass
    try:
        import antenv
        mod = types.ModuleType("antenv.axon_hooks")
        _hook = [None]
        mod.set_axon_ntff_profile_hook = lambda h: _hook.__setitem__(0, h)
        mod.get_axon_ntff_profile_hook = lambda: _hook[0]
        sys.modules["antenv.axon_hooks"] = mod
        antenv.axon_hooks = mod
        if "/root/.axon_site" not in sys.path:
            sys.path.insert(0, "/root/.axon_site")
        from trn_agent_boot.trn_boot import _ntff_profile_via_ctypes
        h = _ntff_profile_via_ctypes("/opt/axon/libaxon_pjrt.so")
        if h is not None:
            mod.set_axon_ntff_profile_hook(h)
    except Exception:
        pass


def kernel(u, A_re, A_im, B_re, B_im, C_re, C_im, D_w, output_bias):
    global LAST_RESULT, _NC_CACHE
    from concourse import bass_utils

    _ensure_axon_hooks()

    u = np.asarray(u, dtype=np.float32)
    shared = _host_prep(
        np.asarray(A_re), np.asarray(A_im), np.asarray(B_re), np.asarray(B_im),
        np.asarray(C_re), np.asarray(C_im), np.asarray(D_w)
    )

    if _NC_CACHE is None:
        _NC_CACHE = _build_nc()
    nc = _NC_CACHE

    in_maps = []
    for k in range(NCORES):
        u_pair = u[BLOCAL * k:BLOCAL * (k + 1)]  # [2, T, IN]
        ut = np.ascontiguousarray(
            u_pair.transpose(2, 0, 1).reshape(128, COLS), dtype=np.float32
        )
        in_maps.append({"blob": np.concatenate([ut, shared], axis=1)})

    res = bass_utils.run_bass_kernel_spmd(nc, in_maps, core_ids=list(range(NCORES)))
    LAST_RESULT = res

    y = np.empty((BATCH, T, OUT), dtype=np.float32)
    for k in range(NCORES):
        yd = res.results[k]["y"]  # [OUT, COLS]
        y[BLOCAL * k:BLOCAL * (k + 1)] = (
            yd.reshape(OUT, BLOCAL, T).transpose(1, 2, 0)
        )
    y += np.asarray(output_bias, dtype=np.float32)
    return y



# revision 3
# speedup vs baseline: 2.6035x; 2.6035x over previous
"""Trainium2 Bass kernel for nn_BaseLinearSSM (chunked hybrid, fp16).

y[b,t] = Re(C x_{t+1}) + D u[b,t] + bias,  x_{t+1} = A x_t + B u_t  (complex A,B,C)

Strategy (L=8 time chunks, Q=T/L=256 chunks):
  Host (fp64): eigendecompose A = V diag(w) V^-1, fold V into B/C:
  Bt = V^-1 B, Ct = C V.  Chunk the recurrence:

    X_q = w^L X_{q-1} + G_q           (coarse, diagonal complex)
    G_q = sum_s w^(L-1-s) Bt u_{qL+s} (chunk input, a stacked matmul)
    y[qL+j] = Re(Ct diag(w^(j+1)) X_{q-1})          (carry, matmul)
            + sum_{s<=j} P_{j-s} u_{qL+s}            (in-chunk, matmul)
    P_k = Re(Ct diag(w^k) Bt),  P_0 += D

  Device (per core, batch-sharded 2 of 16):
    PE (fp16): G matmuls, carry matmuls, in-chunk triangular matmuls
    DVE: modulate e^{-i.phi.q} -> two real tensor_tensor_scans over the
         Q=256 coarse steps only (8x less scan work than per-step) -> demod
    Act: PSUM->SBUF fp16 copies
  All elementwise work is fp16 (DVE 2x mode); matmuls fp16 (PE 1 cyc/row);
  scan state is fp32 internally per the ISA.
"""

import sys

import numpy as np

if "/opt/trn_rl_repo" not in sys.path:
    sys.path.insert(0, "/opt/trn_rl_repo")

BATCH, T, IN, OUT, N = 16, 2048, 128, 128, 512
NCORES = 8
BLOCAL = BATCH // NCORES  # 2
L = 8                     # time-chunk length
Q = T // L                # 256 coarse steps
NT = N // 128             # 4 mode tiles
UCOLS = BLOCAL * T        # 4096, col = b*T + t

# fp16 blob column layout: u | WG (L*2*NT tiles) | PW (L tiles) | cos | sin
#                          | rho | CW (L*2*NT tiles)
WG_TILES = L * 2 * NT     # 64
CW_TILES = L * 2 * NT     # 64
OFF_U = 0
OFF_WG = OFF_U + UCOLS
OFF_PW = OFF_WG + WG_TILES * 128
OFF_COS = OFF_PW + L * 128
OFF_SIN = OFF_COS + NT * Q
OFF_RHO = OFF_SIN + NT * Q
OFF_CW = OFF_RHO + NT * Q
BLOBW = OFF_CW + CW_TILES * 128

LAST_RESULT = None
_NC_CACHE = None


def _build_nc():
    from concourse import bass, mybir
    from concourse import tile

    f16 = mybir.dt.float16
    f32 = mybir.dt.float32
    op = mybir.AluOpType

    nc = bass.Bass("TRN2", target_bir_lowering=False, debug=False)

    blob = nc.dram_tensor("blob", [128, BLOBW], f16, kind="ExternalInput")
    yout = nc.dram_tensor("y", [OUT, UCOLS], f16, kind="ExternalOutput")

    with tile.TileContext(nc) as tc:
        with (
            tc.tile_pool(name="const", bufs=1) as cpool,
            tc.tile_pool(name="gsb", bufs=1) as gpool,
            tc.tile_pool(name="tmp", bufs=4) as tpool,
            tc.tile_pool(name="gh", bufs=3) as hpool,
            tc.tile_pool(name="z", bufs=3) as zpool,
            tc.tile_pool(name="xsh", bufs=1) as xpool,
            tc.tile_pool(name="ysb", bufs=1) as ypool,
            tc.tile_pool(name="pg", bufs=4, space="PSUM") as pgpool,
            tc.tile_pool(name="py", bufs=4, space="PSUM") as pypool,
        ):
            blob_sb = cpool.tile([128, BLOBW], f16)
            # DMA in dependency order: u -> WG|PW -> tables -> CW
            nc.sync.dma_start(blob_sb[:, OFF_U:OFF_WG], blob[:, OFF_U:OFF_WG])
            nc.sync.dma_start(blob_sb[:, OFF_WG:OFF_COS], blob[:, OFF_WG:OFF_COS])
            nc.scalar.dma_start(blob_sb[:, OFF_COS:OFF_CW], blob[:, OFF_COS:OFF_CW])
            nc.scalar.dma_start(blob_sb[:, OFF_CW:BLOBW], blob[:, OFF_CW:BLOBW])

            u_sb = blob_sb[:, OFF_U:OFF_WG]

            def wg(s, p, n):  # G-matmul lhsT tile [128 in, 128 modes]
                i = (s * 2 + p) * NT + n
                return blob_sb[:, OFF_WG + i * 128:OFF_WG + (i + 1) * 128]

            def pw(k):        # in-chunk lhsT tile [128 in, 128 out]
                return blob_sb[:, OFF_PW + k * 128:OFF_PW + (k + 1) * 128]

            def cw(j, p, n):  # carry lhsT tile [128 modes, 128 out]
                i = (j * 2 + p) * NT + n
                return blob_sb[:, OFF_CW + i * 128:OFF_CW + (i + 1) * 128]

            cos_t = [blob_sb[:, OFF_COS + n * Q:OFF_COS + (n + 1) * Q]
                     for n in range(NT)]
            sin_t = [blob_sb[:, OFF_SIN + n * Q:OFF_SIN + (n + 1) * Q]
                     for n in range(NT)]
            rho_t = [blob_sb[:, OFF_RHO + n * Q:OFF_RHO + (n + 1) * Q]
                     for n in range(NT)]

            def ucol(b, s):  # strided u view: chunk tap s of batch b [128, Q]
                return u_sb[:, b * T + s:(b + 1) * T:L]

            # ---- phase A: G matmuls + copies (both batches) ----
            g_sb = [[[None] * NT for _ in range(2)] for _ in range(BLOCAL)]
            for b in range(BLOCAL):
                for p in range(2):
                    for n in range(NT):
                        pg = pgpool.tile([128, Q], f32, tag="pg")
                        for s in range(L):
                            nc.tensor.matmul(
                                pg[:], wg(s, p, n), ucol(b, s),
                                start=(s == 0), stop=(s == L - 1),
                            )
                        gs = gpool.tile([128, Q], f16, tag=f"g{b}{p}{n}")
                        nc.scalar.copy(gs[:], pg[:])
                        g_sb[b][p][n] = gs

            # ---- phase B: modulate -> scan -> demodulate (DVE) ----
            # ---- phase C: carry + in-chunk y matmuls (PE) ----
            xr_sh = [[None] * NT for _ in range(BLOCAL)]
            xi_sh = [[None] * NT for _ in range(BLOCAL)]
            for b in range(BLOCAL):
                for n in range(NT):
                    gr, gi = g_sb[b][0][n], g_sb[b][1][n]
                    ct, st = cos_t[n], sin_t[n]
                    t1 = tpool.tile([128, Q], f16, tag="t1")
                    t2 = tpool.tile([128, Q], f16, tag="t2")
                    nc.vector.tensor_tensor(t1[:], ct, gr[:], op=op.mult)
                    nc.vector.tensor_tensor(t2[:], st, gi[:], op=op.mult)
                    ghr = hpool.tile([128, Q], f16, tag="ghr")
                    nc.vector.tensor_tensor(ghr[:], t1[:], t2[:], op=op.add)
                    t3 = tpool.tile([128, Q], f16, tag="t1")
                    t4 = tpool.tile([128, Q], f16, tag="t2")
                    nc.vector.tensor_tensor(t3[:], ct, gi[:], op=op.mult)
                    nc.vector.tensor_tensor(t4[:], st, gr[:], op=op.mult)
                    ghi = hpool.tile([128, Q], f16, tag="ghi")
                    nc.vector.tensor_tensor(ghi[:], t3[:], t4[:], op=op.subtract)
                    zr = zpool.tile([128, Q], f16, tag="zr")
                    zi = zpool.tile([128, Q], f16, tag="zi")
                    nc.vector.tensor_tensor_scan(
                        zr[:], rho_t[n], ghr[:], 0.0, op0=op.mult, op1=op.add
                    )
                    nc.vector.tensor_tensor_scan(
                        zi[:], rho_t[n], ghi[:], 0.0, op0=op.mult, op1=op.add
                    )
                    # demod into shifted buffers: col 0 = 0 (chunk -1), col
                    # 1+q = X_q; carry for chunk q reads col q.
                    xr = xpool.tile([128, Q + 1], f16, tag=f"xr{b}{n}")
                    xi = xpool.tile([128, Q + 1], f16, tag=f"xi{b}{n}")
                    nc.gpsimd.memset(xr[:, 0:1], 0.0)
                    nc.gpsimd.memset(xi[:, 0:1], 0.0)
                    t5 = tpool.tile([128, Q], f16, tag="t1")
                    t6 = tpool.tile([128, Q], f16, tag="t2")
                    nc.vector.tensor_tensor(t5[:], ct, zr[:], op=op.mult)
                    nc.vector.tensor_tensor(t6[:], st, zi[:], op=op.mult)
                    nc.vector.tensor_tensor(
                        xr[:, 1:Q + 1], t5[:], t6[:], op=op.subtract
                    )
                    t7 = tpool.tile([128, Q], f16, tag="t1")
                    t8 = tpool.tile([128, Q], f16, tag="t2")
                    nc.vector.tensor_tensor(t7[:], st, zr[:], op=op.mult)
                    nc.vector.tensor_tensor(t8[:], ct, zi[:], op=op.mult)
                    nc.vector.tensor_tensor(
                        xi[:, 1:Q + 1], t7[:], t8[:], op=op.add
                    )
                    xr_sh[b][n], xi_sh[b][n] = xr, xi

                # phase C for this batch (emitted after its demods; PE order
                # interleaves with the next batch's G work naturally)
                ysb = ypool.tile([128, T], f16, tag=f"y{b}")
                for j in range(L):
                    py = pypool.tile([128, Q], f32, tag="py")
                    first = True
                    for sp in range(j + 1):
                        nc.tensor.matmul(
                            py[:], pw(j - sp), ucol(b, sp),
                            start=first, stop=False,
                        )
                        first = False
                    for n in range(NT):
                        nc.tensor.matmul(
                            py[:], cw(j, 0, n), xr_sh[b][n][:, 0:Q],
                            start=False, stop=False,
                        )
                        nc.tensor.matmul(
                            py[:], cw(j, 1, n), xi_sh[b][n][:, 0:Q],
                            start=False, stop=(n == NT - 1),
                        )
                    nc.scalar.copy(ysb[:, j:T:L], py[:])
                nc.gpsimd.dma_start(yout[:, b * T:(b + 1) * T], ysb[:])

    _legalize_multi_waits(nc)
    return nc


def _legalize_multi_waits(nc):
    """This walrus build accepts a single sync wait per instruction; split
    any multi-wait instruction into same-engine single-wait NoOps + the
    original carrying the last wait (program order chains them)."""
    import bass_rust
    from concourse import mybir

    uid = [0]
    for fn in nc.m.functions:
        for bb in fn.blocks:
            insts = bb.instructions
            new = []
            changed = False
            for inst in insts:
                si = inst.sync_info
                if si is not None and len(si.on_wait) > 1:
                    waits = list(si.on_wait)
                    for w in waits[:-1]:
                        uid[0] += 1
                        new.append(mybir.InstNoOp(
                            name=f"mwsplit-{uid[0]}",
                            engine=inst.engine,
                            ins=[], outs=[],
                            sync_info=bass_rust.SyncInfo(on_wait=[w], on_update=[]),
                        ))
                    inst.sync_info = bass_rust.SyncInfo(
                        on_wait=[waits[-1]], on_update=list(si.on_update)
                    )
                    changed = True
                new.append(inst)
            if changed:
                bb.instructions = new


def _host_prep(A_re, A_im, B_re, B_im, C_re, C_im, D_w):
    """fp64 eigendecomposition + fp16 weight/table layouts (shared blob
    columns, everything except the per-core u block)."""
    A = A_re.astype(np.float64) + 1j * A_im.astype(np.float64)
    w, V = np.linalg.eig(A)
    Vinv = np.linalg.inv(V)
    Bt = Vinv @ (B_re.astype(np.float64) + 1j * B_im.astype(np.float64))  # [N, IN]
    Ct = (C_re.astype(np.float64) + 1j * C_im.astype(np.float64)) @ V     # [OUT, N]

    wp = w[None, :] ** np.arange(L + 1)[:, None]  # wp[k] = w^k

    parts = []
    # WG tiles: lhsT [128 in, 128 modes] for (s, p, n)
    for s in range(L):
        Ms = wp[L - 1 - s][:, None] * Bt  # [N, IN]
        for comp in (Ms.real, Ms.imag):
            for n in range(NT):
                parts.append(comp[n * 128:(n + 1) * 128, :].T)
    # PW tiles: lhsT [128 in, 128 out]
    Pk = [np.real(Ct @ (wp[k][:, None] * Bt)) for k in range(L)]
    Pk[0] = Pk[0] + D_w.astype(np.float64)
    for k in range(L):
        parts.append(Pk[k].T)
    # tables
    wt = w ** L
    rho = np.abs(wt)
    phi = np.angle(wt)
    qs = np.arange(Q, dtype=np.float64)
    cosT = np.cos(phi[:, None] * qs[None, :]).reshape(NT, 128, Q)
    sinT = np.sin(phi[:, None] * qs[None, :]).reshape(NT, 128, Q)
    rhoT = np.broadcast_to(rho.reshape(NT, 128, 1), (NT, 128, Q))
    for tab in (cosT, sinT, rhoT):
        parts.append(tab.transpose(1, 0, 2).reshape(128, NT * Q))
    # CW tiles: lhsT [128 modes, 128 out] for (j, p, n); p=0 Re, p=1 -Im
    for j in range(L):
        Cj = Ct * wp[j + 1][None, :]  # [OUT, N]
        for comp in (Cj.real, -Cj.imag):
            for n in range(NT):
                parts.append(comp[:, n * 128:(n + 1) * 128].T)
    out = np.concatenate([np.ascontiguousarray(p) for p in parts], axis=1)
    assert out.shape == (128, BLOBW - UCOLS)
    return out.astype(np.float16)


def _ensure_axon_hooks():
    """Provide antenv.axon_hooks if the image lacks it (needed only for
    trace=True NTFF profiling; run path works without)."""
    import types
    try:
        from antenv import axon_hooks  # noqa: F401
        return
    except ImportError:
        pass
    try:
        import antenv
        mod = types.ModuleType("antenv.axon_hooks")
        _hook = [None]
        mod.set_axon_ntff_profile_hook = lambda h: _hook.__setitem__(0, h)
        mod.get_axon_ntff_profile_hook = lambda: _hook[0]
        sys.modules["antenv.axon_hooks"] = mod
        antenv.axon_hooks = mod
        if "/root/.axon_site" not in sys.path:
            sys.path.insert(0, "/root/.axon_site")
        from trn_agent_boot.trn_boot import _ntff_profile_via_ctypes
        h = _ntff_profile_via_ctypes("/opt/axon/libaxon_pjrt.so")
        if h is not None:
            mod.set_axon_ntff_profile_hook(h)
    except Exception:
        pass


def kernel(u, A_re, A_im, B_re, B_im, C_re, C_im, D_w, output_bias):
    global LAST_RESULT, _NC_CACHE
    from concourse import bass_utils

    _ensure_axon_hooks()

    u = np.asarray(u, dtype=np.float32)
    shared = _host_prep(
        np.asarray(A_re), np.asarray(A_im), np.asarray(B_re), np.asarray(B_im),
        np.asarray(C_re), np.asarray(C_im), np.asarray(D_w)
    )

    if _NC_CACHE is None:
        _NC_CACHE = _build_nc()
    nc = _NC_CACHE

    in_maps = []
    for k in range(NCORES):
        u_pair = u[BLOCAL * k:BLOCAL * (k + 1)]  # [2, T, IN]
        ut = np.ascontiguousarray(
            u_pair.transpose(2, 0, 1).reshape(128, UCOLS)
        ).astype(np.float16)
        in_maps.append({"blob": np.concatenate([ut, shared], axis=1)})

    res = bass_utils.run_bass_kernel_spmd(nc, in_maps, core_ids=list(range(NCORES)))
    LAST_RESULT = res

    y = np.empty((BATCH, T, OUT), dtype=np.float32)
    for k in range(NCORES):
        yd = res.results[k]["y"].astype(np.float32)  # [OUT, UCOLS]
        y[BLOCAL * k:BLOCAL * (k + 1)] = (
            yd.reshape(OUT, BLOCAL, T).transpose(1, 2, 0)
        )
    y += np.asarray(output_bias, dtype=np.float32)
    return y


# revision 5
# speedup vs baseline: 4.3108x; 1.6557x over previous
"""Trainium2 Bass kernel for nn_BaseLinearSSM (chunked hybrid, fp16).

y[b,t] = Re(C x_{t+1}) + D u[b,t] + bias,  x_{t+1} = A x_t + B u_t  (complex A,B,C)

Strategy (L=8 time chunks, Q=T/L=256 chunks):
  Host (fp64): eigendecompose A = V diag(w) V^-1, fold V into B/C:
  Bt = V^-1 B, Ct = C V.  Chunk the recurrence:

    X_q = w^L X_{q-1} + G_q           (coarse, diagonal complex)
    G_q = sum_s w^(L-1-s) Bt u_{qL+s} (chunk input, a stacked matmul)
    y[qL+j] = Re(Ct diag(w^(j+1)) X_{q-1})          (carry, matmul)
            + sum_{s<=j} P_{j-s} u_{qL+s}            (in-chunk, matmul)
    P_k = Re(Ct diag(w^k) Bt),  P_0 += D

  Device (per core, batch-sharded 2 of 16):
    PE (fp16): G matmuls, carry matmuls, in-chunk triangular matmuls
    DVE: modulate e^{-i.phi.q} -> two real tensor_tensor_scans over the
         Q=256 coarse steps only (8x less scan work than per-step) -> demod
    Act: PSUM->SBUF fp16 copies

  Layout notes: u and y are chunk-major on device (col = b|s|q resp.
  b|j|q, host pre/post-permutes) so every matmul rhs and every output DMA
  is contiguous; both batches share one [128, 2Q] tile per (part, mode
  tile) so each weight LDW feeds two matmuls, and the coarse scans run
  segmented (rho zeroed at the batch-boundary column).
"""

import sys

import numpy as np

if "/opt/trn_rl_repo" not in sys.path:
    sys.path.insert(0, "/opt/trn_rl_repo")

BATCH, T, IN, OUT, N = 16, 2048, 128, 128, 512
NCORES = 8
BLOCAL = BATCH // NCORES  # 2
L = 8                     # time-chunk length
Q = T // L                # 256 coarse steps
Q2 = BLOCAL * Q           # 512 coarse cols, col = b*Q + q
NT = N // 128             # 4 mode tiles
UCOLS = BLOCAL * T        # 4096; u col = (b*L + s)*Q + q  <=>  u[b, q*L+s]

# fp16 blob column layout: u | WG | PW | cos | sin | rho | CW
WG_TILES = 2 * NT * L     # 64, index (n, p, s)
CW_TILES = L * 2 * NT     # 64, index (j, p, n)
OFF_U = 0
OFF_WG = OFF_U + UCOLS
OFF_PW = OFF_WG + WG_TILES * 128
OFF_COS = OFF_PW + L * 128
OFF_SIN = OFF_COS + NT * Q2
OFF_RHO = OFF_SIN + NT * Q2
OFF_CW = OFF_RHO + NT * Q2
BLOBW = OFF_CW + CW_TILES * 128

LAST_RESULT = None
_NC_CACHE = None


def _build_nc():
    from concourse import bass, mybir
    from concourse import tile

    f16 = mybir.dt.float16
    f32 = mybir.dt.float32
    op = mybir.AluOpType

    nc = bass.Bass("TRN2", target_bir_lowering=False, debug=False)

    blob = nc.dram_tensor("blob", [128, BLOBW], f16, kind="ExternalInput")
    # y col = (b*L + j)*Q + q  <=>  y[b, q*L+j]
    yout = nc.dram_tensor("y", [OUT, UCOLS], f16, kind="ExternalOutput")

    with tile.TileContext(nc) as tc:
        with (
            tc.tile_pool(name="const", bufs=1) as cpool,
            tc.tile_pool(name="gsb", bufs=1) as gpool,
            tc.tile_pool(name="tmp", bufs=4) as tpool,
            tc.tile_pool(name="gh", bufs=3) as hpool,
            tc.tile_pool(name="z", bufs=3) as zpool,
            tc.tile_pool(name="xsh", bufs=1) as xpool,
            tc.tile_pool(name="ysb", bufs=1) as ypool,
            tc.tile_pool(name="pg", bufs=4, space="PSUM") as pgpool,
            tc.tile_pool(name="py", bufs=4, space="PSUM") as pypool,
        ):
            blob_sb = cpool.tile([128, BLOBW], f16)
            # DMA pieces in consumption order; two issuing queues.
            WG_HALF = OFF_WG + WG_TILES * 64
            for a, bnd in [(OFF_U, OFF_U + T), (OFF_U + T, OFF_WG),
                           (OFF_WG, WG_HALF), (WG_HALF, OFF_PW)]:
                nc.sync.dma_start(blob_sb[:, a:bnd], blob[:, a:bnd])
            for a, bnd in [(OFF_PW, OFF_CW), (OFF_CW, BLOBW)]:
                nc.scalar.dma_start(blob_sb[:, a:bnd], blob[:, a:bnd])

            def wg(n, p, s):  # G-matmul lhsT tile [128 in, 128 modes]
                i = ((n * 2 + p) * L + s)
                return blob_sb[:, OFF_WG + i * 128:OFF_WG + (i + 1) * 128]

            def pw(k):        # in-chunk lhsT tile [128 in, 128 out]
                return blob_sb[:, OFF_PW + k * 128:OFF_PW + (k + 1) * 128]

            def cw(j, p, n):  # carry lhsT tile [128 modes, 128 out]
                i = (j * 2 + p) * NT + n
                return blob_sb[:, OFF_CW + i * 128:OFF_CW + (i + 1) * 128]

            cos_t = [blob_sb[:, OFF_COS + n * Q2:OFF_COS + (n + 1) * Q2]
                     for n in range(NT)]
            sin_t = [blob_sb[:, OFF_SIN + n * Q2:OFF_SIN + (n + 1) * Q2]
                     for n in range(NT)]
            rho_t = [blob_sb[:, OFF_RHO + n * Q2:OFF_RHO + (n + 1) * Q2]
                     for n in range(NT)]

            def ucol(b, s):  # contiguous chunk-tap slice [128, Q]
                a = OFF_U + (b * L + s) * Q
                return blob_sb[:, a:a + Q]

            # ---- phase A: G matmuls (per-batch psum, paired sbuf tile) ----
            g_sb = [[None] * 2 for _ in range(NT)]
            for n in range(NT):
                for p in range(2):
                    gs = gpool.tile([128, Q2], f16, tag=f"g{p}{n}")
                    for b in range(BLOCAL):
                        pg = pgpool.tile([128, Q], f32, tag="pg")
                        for s in range(L):
                            nc.tensor.matmul(
                                pg[:], wg(n, p, s), ucol(b, s),
                                start=(s == 0), stop=(s == L - 1),
                            )
                        nc.scalar.copy(gs[:, b * Q:(b + 1) * Q], pg[:])
                    g_sb[n][p] = gs

            # ---- phase B: modulate -> segmented scan -> demodulate ----
            xr_sh = [None] * NT
            xi_sh = [None] * NT
            for n in range(NT):
                gr, gi = g_sb[n][0], g_sb[n][1]
                ct, st = cos_t[n], sin_t[n]
                t1 = tpool.tile([128, Q2], f16, tag="t1")
                t2 = tpool.tile([128, Q2], f16, tag="t2")
                nc.vector.tensor_tensor(t1[:], ct, gr[:], op=op.mult)
                nc.vector.tensor_tensor(t2[:], st, gi[:], op=op.mult)
                ghr = hpool.tile([128, Q2], f16, tag="ghr")
                nc.vector.tensor_tensor(ghr[:], t1[:], t2[:], op=op.add)
                t3 = tpool.tile([128, Q2], f16, tag="t1")
                t4 = tpool.tile([128, Q2], f16, tag="t2")
                nc.vector.tensor_tensor(t3[:], ct, gi[:], op=op.mult)
                nc.vector.tensor_tensor(t4[:], st, gr[:], op=op.mult)
                ghi = hpool.tile([128, Q2], f16, tag="ghi")
                nc.vector.tensor_tensor(ghi[:], t3[:], t4[:], op=op.subtract)
                zr = zpool.tile([128, Q2], f16, tag="zr")
                zi = zpool.tile([128, Q2], f16, tag="zi")
                # rho has col Q zeroed -> state resets at the b=1 boundary
                nc.vector.tensor_tensor_scan(
                    zr[:], rho_t[n], ghr[:], 0.0, op0=op.mult, op1=op.add
                )
                nc.vector.tensor_tensor_scan(
                    zi[:], rho_t[n], ghi[:], 0.0, op0=op.mult, op1=op.add
                )
                # demod into shifted buffers: per batch, col b*(Q+1) = 0
                # (chunk -1), col b*(Q+1)+1+q = X_q; carry reads cols
                # [b*(Q+1), b*(Q+1)+Q).
                xr = xpool.tile([128, 2 * (Q + 1)], f16, tag=f"xr{n}")
                xi = xpool.tile([128, 2 * (Q + 1)], f16, tag=f"xi{n}")
                t5 = tpool.tile([128, Q2], f16, tag="t1")
                t6 = tpool.tile([128, Q2], f16, tag="t2")
                nc.vector.tensor_tensor(t5[:], ct, zr[:], op=op.mult)
                nc.vector.tensor_tensor(t6[:], st, zi[:], op=op.mult)
                t7 = tpool.tile([128, Q2], f16, tag="t7")
                t8 = tpool.tile([128, Q2], f16, tag="t8")
                nc.vector.tensor_tensor(t7[:], st, zr[:], op=op.mult)
                nc.vector.tensor_tensor(t8[:], ct, zi[:], op=op.mult)
                for b in range(BLOCAL):
                    c0 = b * (Q + 1)
                    nc.gpsimd.memset(xr[:, c0:c0 + 1], 0.0)
                    nc.gpsimd.memset(xi[:, c0:c0 + 1], 0.0)
                    nc.vector.tensor_tensor(
                        xr[:, c0 + 1:c0 + 1 + Q],
                        t5[:, b * Q:(b + 1) * Q], t6[:, b * Q:(b + 1) * Q],
                        op=op.subtract,
                    )
                    nc.vector.tensor_tensor(
                        xi[:, c0 + 1:c0 + 1 + Q],
                        t7[:, b * Q:(b + 1) * Q], t8[:, b * Q:(b + 1) * Q],
                        op=op.add,
                    )
                xr_sh[n], xi_sh[n] = xr, xi

            # ---- phase C: in-chunk + carry y matmuls ----
            ysb = ypool.tile([128, UCOLS], f16, tag="y")
            for j in range(L):
                for b in range(BLOCAL):
                    py = pypool.tile([128, Q], f32, tag="py")
                    for sp in range(j + 1):
                        nc.tensor.matmul(
                            py[:], pw(j - sp), ucol(b, sp),
                            start=(sp == 0), stop=False,
                        )
                    for p in range(2):
                        xs = xr_sh if p == 0 else xi_sh
                        for n in range(NT):
                            last = (p == 1 and n == NT - 1)
                            c0 = b * (Q + 1)
                            nc.tensor.matmul(
                                py[:], cw(j, p, n), xs[n][:, c0:c0 + Q],
                                start=False, stop=last,
                            )
                    nc.scalar.copy(
                        ysb[:, (b * L + j) * Q:(b * L + j + 1) * Q], py[:]
                    )
            for b in range(BLOCAL):
                nc.gpsimd.dma_start(
                    yout[:, b * T:(b + 1) * T],
                    ysb[:, b * T:(b + 1) * T],
                )

    _legalize_multi_waits(nc)
    return nc


def _legalize_multi_waits(nc):
    """This walrus build accepts a single sync wait per instruction; split
    any multi-wait instruction into same-engine single-wait NoOps + the
    original carrying the last wait (program order chains them)."""
    import bass_rust
    from concourse import mybir

    uid = [0]
    for fn in nc.m.functions:
        for bb in fn.blocks:
            insts = bb.instructions
            new = []
            changed = False
            for inst in insts:
                si = inst.sync_info
                if si is not None and len(si.on_wait) > 1:
                    waits = list(si.on_wait)
                    for w in waits[:-1]:
                        uid[0] += 1
                        new.append(mybir.InstNoOp(
                            name=f"mwsplit-{uid[0]}",
                            engine=inst.engine,
                            ins=[], outs=[],
                            sync_info=bass_rust.SyncInfo(on_wait=[w], on_update=[]),
                        ))
                    inst.sync_info = bass_rust.SyncInfo(
                        on_wait=[waits[-1]], on_update=list(si.on_update)
                    )
                    changed = True
                new.append(inst)
            if changed:
                bb.instructions = new


def _host_prep(A_re, A_im, B_re, B_im, C_re, C_im, D_w):
    """fp64 eigendecomposition + fp16 weight/table layouts (shared blob
    columns, everything except the per-core u block)."""
    A = A_re.astype(np.float64) + 1j * A_im.astype(np.float64)
    w, V = np.linalg.eig(A)
    Vinv = np.linalg.inv(V)
    Bt = Vinv @ (B_re.astype(np.float64) + 1j * B_im.astype(np.float64))  # [N, IN]
    Ct = (C_re.astype(np.float64) + 1j * C_im.astype(np.float64)) @ V     # [OUT, N]

    wp = w[None, :] ** np.arange(L + 1)[:, None]  # wp[k] = w^k

    parts = []
    # WG tiles: lhsT [128 in, 128 modes] for (n, p, s)
    Ms = [wp[L - 1 - s][:, None] * Bt for s in range(L)]  # [N, IN]
    for n in range(NT):
        for p in range(2):
            for s in range(L):
                comp = Ms[s].real if p == 0 else Ms[s].imag
                parts.append(comp[n * 128:(n + 1) * 128, :].T)
    # PW tiles: lhsT [128 in, 128 out]
    Pk = [np.real(Ct @ (wp[k][:, None] * Bt)) for k in range(L)]
    Pk[0] = Pk[0] + D_w.astype(np.float64)
    for k in range(L):
        parts.append(Pk[k].T)
    # tables, cols (b, q); rho zeroed at the batch boundary col Q
    wt = w ** L
    rho = np.abs(wt)
    phi = np.angle(wt)
    qs = np.arange(Q, dtype=np.float64)
    cos1 = np.cos(phi[:, None] * qs[None, :])
    sin1 = np.sin(phi[:, None] * qs[None, :])
    rho1 = np.broadcast_to(rho[:, None], (N, Q)).copy()
    rho2 = np.concatenate([rho1, rho1], axis=1)
    rho2[:, Q] = 0.0
    for tab in (np.concatenate([cos1, cos1], 1), np.concatenate([sin1, sin1], 1),
                rho2):
        parts.append(tab.reshape(NT, 128, Q2).transpose(1, 0, 2).reshape(128, NT * Q2))
    # CW tiles: lhsT [128 modes, 128 out] for (j, p, n); p=0 Re, p=1 -Im
    for j in range(L):
        Cj = Ct * wp[j + 1][None, :]  # [OUT, N]
        for comp in (Cj.real, -Cj.imag):
            for n in range(NT):
                parts.append(comp[:, n * 128:(n + 1) * 128].T)
    out = np.concatenate([np.ascontiguousarray(p) for p in parts], axis=1)
    assert out.shape == (128, BLOBW - UCOLS)
    return out.astype(np.float16)


def _ensure_axon_hooks():
    """Provide antenv.axon_hooks if the image lacks it (needed only for
    trace=True NTFF profiling; run path works without)."""
    import types
    try:
        from antenv import axon_hooks  # noqa: F401
        return
    except ImportError:
        pass
    try:
        import antenv
        mod = types.ModuleType("antenv.axon_hooks")
        _hook = [None]
        mod.set_axon_ntff_profile_hook = lambda h: _hook.__setitem__(0, h)
        mod.get_axon_ntff_profile_hook = lambda: _hook[0]
        sys.modules["antenv.axon_hooks"] = mod
        antenv.axon_hooks = mod
        if "/root/.axon_site" not in sys.path:
            sys.path.insert(0, "/root/.axon_site")
        from trn_agent_boot.trn_boot import _ntff_profile_via_ctypes
        h = _ntff_profile_via_ctypes("/opt/axon/libaxon_pjrt.so")
        if h is not None:
            mod.set_axon_ntff_profile_hook(h)
    except Exception:
        pass


def kernel(u, A_re, A_im, B_re, B_im, C_re, C_im, D_w, output_bias):
    global LAST_RESULT, _NC_CACHE
    from concourse import bass_utils

    _ensure_axon_hooks()

    u = np.asarray(u, dtype=np.float32)
    shared = _host_prep(
        np.asarray(A_re), np.asarray(A_im), np.asarray(B_re), np.asarray(B_im),
        np.asarray(C_re), np.asarray(C_im), np.asarray(D_w)
    )

    if _NC_CACHE is None:
        _NC_CACHE = _build_nc()
    nc = _NC_CACHE

    in_maps = []
    for k in range(NCORES):
        u_pair = u[BLOCAL * k:BLOCAL * (k + 1)]  # [2, T, IN]
        # chunk-tap-major: col (b, s, q) = u[b, q*L+s, i]
        ut = np.ascontiguousarray(
            u_pair.reshape(BLOCAL, Q, L, IN).transpose(3, 0, 2, 1)
            .reshape(128, UCOLS)
        ).astype(np.float16)
        in_maps.append({"blob": np.concatenate([ut, shared], axis=1)})

    res = bass_utils.run_bass_kernel_spmd(nc, in_maps, core_ids=list(range(NCORES)))
    LAST_RESULT = res

    y = np.empty((BATCH, T, OUT), dtype=np.float32)
    for k in range(NCORES):
        yd = res.results[k]["y"].astype(np.float32)  # [OUT, (b, j, q)]
        # y[b, q*L+j, o] = yd[o, (b*L+j)*Q+q]
        y[BLOCAL * k:BLOCAL * (k + 1)] = (
            yd.reshape(OUT, BLOCAL, L, Q).transpose(1, 3, 2, 0)
            .reshape(BLOCAL, T, OUT)
        )
    y += np.asarray(output_bias, dtype=np.float32)
    return y
